# revision 10
# baseline (speedup 1.0000x reference)
"""GCN message-passing kernel (nn_Encoder_21646635172361) for 8 Trainium2 cores.

Math (reference):
    h   = x @ W.T                     [N,H]
    A~  = adjacency + self loops, symmetric-normalized: norm(r,c) = dinv[r]*dinv[c]
    out = PReLU(A~ @ h + b, alpha)

Key algebraic restructure: aggregation commutes with the linear transform,
    A~ @ (x W.T) = (A~ @ x) W.T
so we aggregate F=128-wide rows (4x less gather traffic than H=512).
    agg[c] = dinv[c] * ( sum_{r->c} dinv[r]*x[r] + dinv[c]*x[c] )
Host prescales xs = dinv*x (fp16), so per-edge messages are plain rows of xs
and the scatter-add becomes binary one-hot matmuls on the TensorEngine.
Self-loops are NOT gathered: their xs rows are shipped per-core in local
order (xloc) and folded into each window's chain via one identity mask;
the epilogue's dinv[dst] completes dinv^2.

Distribution: destination nodes sharded round-robin (dst % 8) across the 8
cores; xs replicated; each core gathers the source rows for its own edges
(DistGNN-style edge partition, no collectives needed).

Performance notes (v4, measured on HW):
 - dma_gather desc-gen runs on the Q7 core pair (2q, 2q+1) of its SWDGE
   queue q: ~8.6us per 1024-idx call per queue, but the 4 queues work
   CONCURRENTLY.  Round-robin chunks across all 4 queues -> ~2.2us/call
   effective (~2.1 ns/idx), 4x the single-queue baseline.  All gather
   chunks stay SBUF-resident so the stream never stalls on recycling.
 - TRN2 PE accumulation chains into a single PSUM bank stall ~800ns/matmul
   on the bank read-modify-write; interleaving 4 windows' chains across 4
   PSUM banks runs at ~136ns per matmul.  WIN=128 keeps the one-hot rhs
   small (PE 256cyc/tile) and the epilogue one block per window.
 - One-hots are built one DVE is_equal per 8-tile chunk (materialized
   repeating iota, fp16 dlc broadcast via stride-0 AP) instead of one op
   per tile.
 - Uniform alpha (the PReLU default init) collapses the epilogue to a
   single matmul + one scalar Prelu activation per window.
 - Output is written fp16 (half the HBM write traffic); host converts to
   fp32.  absmax-rel error stays ~5e-4, well under the 2e-2 gate.
"""

import os
import time
from contextlib import ExitStack

import numpy as np

N, F, H = 50000, 128, 512
NC_CORES = 8
ND = N // NC_CORES            # 6250 local dst nodes per core
WIN = 128                     # dst window width (one PE tile of dst rows)
NW = (ND + WIN - 1) // WIN    # 49 windows
NDP = NW * WIN                # 6272 padded local dst rows
SPLIT = 32768                 # int16 gather index split point
CH = 1024                     # gather chunk size (edges per dma_gather; >1024 fails on HW)
TILE = 128                    # edges per PE tile
CHB = CH // TILE              # tiles per chunk (8)
GRP = 4                       # windows interleaved per PSUM-bank group

# Results of the last kernel() call (for test.py introspection)
last_run_info = {}


def _plan(edge_index, x, W, alpha):
    """Host-side graph partition + input prep. Returns per-core arrays +
    shared tile structure (uniform across cores, required for SPMD)."""
    src = np.asarray(edge_index[0], dtype=np.int64)
    dst = np.asarray(edge_index[1], dtype=np.int64)

    # degrees include self-loops (reference adds them)
    deg = np.bincount(dst, minlength=N) + 1
    dinv = (1.0 / np.sqrt(deg.astype(np.float64))).astype(np.float32)

    # xs rows serve both edge messages (epilogue adds dinv[dst]) and
    # self-loops (same epilogue factor completes dinv[d]^2).
    xs = (dinv[:, None] * x).astype(np.float16)           # [N, F]

    core = (dst % NC_CORES).astype(np.int64)
    loc = dst // NC_CORES
    win = loc // WIN
    dloc = (loc % WIN).astype(np.float32)
    low = src < SPLIT

    # group edges per (core, window, stream)
    grp = {}
    for k in range(NC_CORES):
        mk = core == k
        s_k, w_k, dl_k, lo_k = src[mk], win[mk], dloc[mk], low[mk]
        for w in range(NW):
            mw = w_k == w
            s_w, dl_w, lo_w = s_k[mw], dl_k[mw], lo_k[mw]
            grp[(k, w, 0)] = (s_w[lo_w], dl_w[lo_w])
            grp[(k, w, 1)] = (s_w[~lo_w] - SPLIT, dl_w[~lo_w])

    # uniform tile counts across cores
    T = np.zeros((2, NW), dtype=np.int64)
    for st in range(2):
        for w in range(NW):
            cnt = max(len(grp[(k, w, st)][0]) for k in range(NC_CORES))
            T[st, w] = (cnt + TILE - 1) // TILE
    tile_start = np.zeros((2, NW), dtype=np.int64)
    tile_start[0, 1:] = np.cumsum(T[0])[:-1]
    tile_start[1, 1:] = np.cumsum(T[1])[:-1]
    NT = [int(T[0].sum()), int(T[1].sum())]

    per_core = []
    for k in range(NC_CORES):
        core_dat = {}
        for st in range(2):
            nt = NT[st]
            idx = np.zeros(nt * TILE, dtype=np.int16)      # pad -> row 0 (harmless)
            dlc = np.full(nt * TILE, -1.0, dtype=np.float16)  # pad -> no one-hot match
            for w in range(NW):
                s_w, dl_w = grp[(k, w, st)]
                o = tile_start[st, w] * TILE
                n = len(s_w)
                idx[o:o + n] = s_w.astype(np.int16)
                dlc[o:o + n] = dl_w.astype(np.float16)
            # wrapped int16 index layout: [p, j] = idx[j*16 + p%16], replicated
            wrapped = idx.reshape(-1, 16).T            # [16, nt*8]
            wrapped = np.tile(wrapped, (8, 1)).copy()  # [128, nt*8]
            core_dat[("idx", st)] = wrapped
            core_dat[("dlc", st)] = dlc.reshape(nt, TILE).T.copy()  # [128, nt] fp16
        # local dst rows (xloc) and dinv tables in window-local order
        g = np.arange(NDP, dtype=np.int64) * NC_CORES + k
        valid = np.arange(NDP) < ND
        gc = np.minimum(g, N - 1)
        xl = np.where(valid[:, None], xs[gc], np.float16(0.0))      # [NDP, F]
        core_dat["xloc"] = np.ascontiguousarray(xl)
        dv = np.where(valid, dinv[gc], 0.0).astype(np.float32)
        core_dat["dv"] = dv.reshape(NW, TILE).T.copy()              # [128, NW]
        core_dat["dvn"] = (-core_dat["dv"]).copy()
        per_core.append(core_dat)

    return per_core, T, tile_start, NT, xs


def _build_program(T, tile_start, NT, fast_path, uniform_alpha, alpha0):
    import concourse.mybir as mybir
    import concourse.tile as tile
    from concourse import bacc

    f32 = mybir.dt.float32
    fp16 = mybir.dt.float16
    i16 = mybir.dt.int16
    Alu = mybir.AluOpType
    Act = mybir.ActivationFunctionType

    nc = bacc.Bacc("TRN2", target_bir_lowering=False, debug=False,
                   num_devices=NC_CORES, num_swdge_queues=4)

    xs_d = nc.dram_tensor("xs", [N, F], fp16, kind="ExternalInput").ap()
    xloc_d = nc.dram_tensor("xloc", [NDP, F], fp16, kind="ExternalInput").ap()
    wt_d = nc.dram_tensor("w_t", [F, H], fp16, kind="ExternalInput").ap()
    w1t_d = nc.dram_tensor("w1_t", [F, H], fp16, kind="ExternalInput").ap()
    idx_d = [nc.dram_tensor(f"idx{st}", [128, NT[st] * 8], i16,
                            kind="ExternalInput").ap() for st in range(2)]
    dlc_d = [nc.dram_tensor(f"dlc{st}", [128, NT[st]], fp16,
                            kind="ExternalInput").ap() for st in range(2)]
    dv_d = nc.dram_tensor("dv", [128, NW], f32, kind="ExternalInput").ap()
    dvn_d = nc.dram_tensor("dvn", [128, NW], f32, kind="ExternalInput").ap()
    if not fast_path:
        arow_d = nc.dram_tensor("alpha_row", [1, H], f32, kind="ExternalInput").ap()
        brow_d = nc.dram_tensor("b_row", [1, H], f32, kind="ExternalInput").ap()
    out_d = nc.dram_tensor("out", [NDP, H], fp16, kind="ExternalOutput").ap()

    xs_lo = xs_d[0:SPLIT, :]
    xs_hi = xs_d[SPLIT:N, :]
    x_in = [xs_lo, xs_hi]

    n_chunks = [(NT[st] * TILE + CH - 1) // CH for st in range(2)]
    n_chunks_tot = n_chunks[0] + n_chunks[1]

    with tile.TileContext(nc) as tc, ExitStack() as ctx:
        cpool = ctx.enter_context(tc.tile_pool(name="const", bufs=1))
        # all gather chunks stay resident: no recycle stalls on the gather
        gxpool = ctx.enter_context(tc.tile_pool(name="gx", bufs=n_chunks_tot))
        xlpool = ctx.enter_context(tc.tile_pool(name="xl", bufs=10))
        ohpool = ctx.enter_context(tc.tile_pool(name="oh", bufs=10))
        aggpool = ctx.enter_context(tc.tile_pool(name="aggs", bufs=2))
        eppool = ctx.enter_context(tc.tile_pool(name="ep", bufs=2))
        ps_agg = ctx.enter_context(tc.tile_pool(name="ps_agg", bufs=1, space="PSUM"))
        ps_out = ctx.enter_context(tc.tile_pool(name="ps_out", bufs=2, space="PSUM"))

        # ---- one-time loads (idx tables first: the gather stream waits on them) ----
        idx_sb, dlc_sb = [], []
        for st in range(2):
            t = cpool.tile([128, NT[st] * 8], i16, tag=f"idx{st}")
            nc.sync.dma_start(t[:], idx_d[st])
            idx_sb.append(t)
        for st in range(2):
            t = cpool.tile([128, NT[st]], fp16, tag=f"dlc{st}")
            nc.sync.dma_start(t[:], dlc_d[st])
            dlc_sb.append(t)
        dv_sb = cpool.tile([128, NW], f32)
        nc.sync.dma_start(dv_sb[:], dv_d)
        dvn_sb = cpool.tile([128, NW], f32)
        nc.sync.dma_start(dvn_sb[:], dvn_d)
        wt_sb = cpool.tile([F, H], fp16)
        nc.sync.dma_start(wt_sb[:], wt_d)
        if fast_path and not uniform_alpha:
            w1t_sb = cpool.tile([F, H], fp16)
            nc.sync.dma_start(w1t_sb[:], w1t_d)

        # repeating iota [128, CHB, WIN]: value = free_idx % WIN (per chunk slot)
        iota_rep = cpool.tile([128, CHB, WIN], fp16)
        nc.gpsimd.iota(iota_rep[:], pattern=[[0, CHB], [1, WIN]], base=0,
                       channel_multiplier=0,
                       allow_small_or_imprecise_dtypes=True)
        iota_col = cpool.tile([128, 1], f32)
        nc.gpsimd.iota(iota_col[:], pattern=[[1, 1]], base=0,
                       channel_multiplier=1,
                       allow_small_or_imprecise_dtypes=True)
        # identity mask for the self-loop tile of each window
        selfoh = cpool.tile([128, WIN], fp16)
        nc.vector.tensor_scalar(selfoh[:], iota_rep[:, 0, :], iota_col[:], None,
                                op0=Alu.is_equal)

        if not fast_path:
            ones_sb = cpool.tile([1, 128], f32)
            nc.vector.memset(ones_sb[:], 1.0)
            arow_sb = cpool.tile([1, H], f32)
            nc.sync.dma_start(arow_sb[:], arow_d)
            brow_sb = cpool.tile([1, H], f32)
            nc.sync.dma_start(brow_sb[:], brow_d)
            arep_ps = ps_out.tile([128, H], f32, tag="brd")
            nc.tensor.matmul(arep_ps[:], lhsT=ones_sb[:], rhs=arow_sb[:],
                             start=True, stop=True)
            arep_sb = cpool.tile([128, H], f32)
            nc.scalar.copy(arep_sb[:], arep_ps[:])
            brep_ps = ps_out.tile([128, H], f32, tag="brd")
            nc.tensor.matmul(brep_ps[:], lhsT=ones_sb[:], rhs=brow_sb[:],
                             start=True, stop=True)
            brep_sb = cpool.tile([128, H], f32)
            nc.scalar.copy(brep_sb[:], brep_ps[:])

        # ---- main loop ----
        gx_tiles = [[None] * n_chunks[0], [None] * n_chunks[1]]
        oh_tiles = [[None] * n_chunks[0], [None] * n_chunks[1]]
        q_counter = [0]

        def chunk_tile(st, c):
            if gx_tiles[st][c] is None:
                num = min(CH, NT[st] * TILE - c * CH)
                nblk = num // TILE
                gx = gxpool.tile([128, CHB, TILE], fp16, tag="gx")
                nc.gpsimd.dma_gather(
                    out_ap=gx[:, 0:nblk, :],
                    in_ap=x_in[st],
                    idxs_ap=idx_sb[st][:, c * (CH // 16): c * (CH // 16) + num // 16],
                    num_idxs=num,
                    num_idxs_reg=num,
                    elem_size=F,
                    queue_num=q_counter[0] % 4,
                )
                q_counter[0] += 1
                gx_tiles[st][c] = gx
            return gx_tiles[st][c]

        def oh_chunk_tile(st, c):
            # one is_equal per chunk: materialized repeating iota vs the dlc
            # column broadcast over the WIN positions.
            if oh_tiles[st][c] is None:
                nblk = min(CHB, NT[st] - c * CHB)
                oh = ohpool.tile([128, CHB, WIN], fp16, tag="oh")
                dlc_b = (dlc_sb[st][:, c * CHB: c * CHB + nblk]
                         .unsqueeze(2).to_broadcast([128, nblk, WIN]))
                nc.vector.tensor_tensor(oh[:, 0:nblk, :],
                                        iota_rep[:, 0:nblk, :], dlc_b,
                                        op=Alu.is_equal)
                oh_tiles[st][c] = oh
            return oh_tiles[st][c]

        def epilogue(w, pagg):
            agg_sb = aggpool.tile([128, WIN], fp16, tag="aggs")
            nc.scalar.copy(agg_sb[:], pagg[:])
            dv_col = dv_sb[:, w:w + 1]
            ps0 = ps_out.tile([128, H], f32, tag="ps0")
            nc.tensor.matmul(ps0[:], lhsT=agg_sb[:], rhs=wt_sb[:],
                             start=True, stop=True)
            if uniform_alpha:
                # out = PReLU(dv*z0; alpha0): single activation, no z1
                outt = eppool.tile([128, H], fp16, tag="outt")
                nc.scalar.activation(outt[:], ps0[:], Act.Prelu,
                                     scale=dv_col, alpha=float(alpha0))
            elif fast_path:
                # out = relu(dv*z0) - relu(-dv*z1), z1 = agg @ (alpha W)^T
                ps1 = ps_out.tile([128, H], f32, tag="ps1")
                nc.tensor.matmul(ps1[:], lhsT=agg_sb[:], rhs=w1t_sb[:],
                                 start=True, stop=True)
                pos = eppool.tile([128, H], f32, tag="pos")
                nc.scalar.activation(pos[:], ps0[:], Act.Relu, scale=dv_col)
                neg = eppool.tile([128, H], f32, tag="neg")
                nc.scalar.activation(neg[:], ps1[:], Act.Relu,
                                     scale=dvn_sb[:, w:w + 1])
                outt = eppool.tile([128, H], fp16, tag="outt")
                nc.vector.tensor_tensor(outt[:], pos[:], neg[:],
                                        op=Alu.subtract)
            else:
                # general: v = dv*z0 + b; out = relu(v) + alpha*min(v,0)
                vb = eppool.tile([128, H], f32, tag="vb")
                nc.vector.tensor_scalar(vb[:], ps0[:], dv_col, None,
                                        op0=Alu.mult)
                vb2 = eppool.tile([128, H], f32, tag="vb2")
                nc.vector.tensor_tensor(vb2[:], vb[:], brep_sb[:],
                                        op=Alu.add)
                pos = eppool.tile([128, H], f32, tag="pos")
                nc.scalar.activation(pos[:], vb2[:], Act.Relu)
                neg = eppool.tile([128, H], f32, tag="neg")
                nc.vector.tensor_scalar(neg[:], vb2[:], 0.0, None,
                                        op0=Alu.min)
                nega = eppool.tile([128, H], f32, tag="nega")
                nc.vector.tensor_tensor(nega[:], neg[:], arep_sb[:],
                                        op=Alu.mult)
                outt = eppool.tile([128, H], fp16, tag="outt")
                nc.vector.tensor_tensor(outt[:], pos[:], nega[:],
                                        op=Alu.add)
            nc.sync.dma_start(out_d[w * WIN:(w + 1) * WIN, :], outt[:])

        # windows in groups of GRP; interleave matmul emission across the
        # group so consecutive PE instructions hit different PSUM banks.
        for g0 in range(0, NW, GRP):
            wins = list(range(g0, min(g0 + GRP, NW)))
            paggs = {}
            mm_count = {}
            n_mm = {}
            for j, w in enumerate(wins):
                paggs[w] = ps_agg.tile([128, WIN], f32, name=f"pagg{j}",
                                       tag=f"pagg{j}")
                mm_count[w] = 0
                n_mm[w] = 1 + int(T[0, w] + T[1, w])

            # self-loop matmul first (starts each window's chain)
            for w in wins:
                xl = xlpool.tile([128, F], fp16, tag="xl")
                r0 = w * WIN
                nc.sync.dma_start(xl[:], xloc_d[r0:r0 + WIN, :])
                nc.tensor.matmul(paggs[w][:], lhsT=xl[:], rhs=selfoh[:],
                                 start=True,
                                 stop=(mm_count[w] == n_mm[w] - 1))
                mm_count[w] += 1

            # round-robin the gather tiles across the group's windows
            tl = {w: [(st, t) for st in range(2) for t in range(int(T[st, w]))]
                  for w in wins}
            pos_i = {w: 0 for w in wins}
            remaining = sum(len(v) for v in tl.values())
            while remaining:
                for w in wins:
                    if pos_i[w] >= len(tl[w]):
                        continue
                    st, t = tl[w][pos_i[w]]
                    pos_i[w] += 1
                    remaining -= 1
                    gt = int(tile_start[st, w]) + t
                    c, blk = divmod(gt, CHB)
                    gx = chunk_tile(st, c)
                    oh = oh_chunk_tile(st, c)
                    nc.tensor.matmul(
                        paggs[w][:],
                        lhsT=gx[:, blk:blk + 1, :],
                        rhs=oh[:, blk, :],
                        start=(mm_count[w] == 0),
                        stop=(mm_count[w] == n_mm[w] - 1),
                    )
                    mm_count[w] += 1

            for w in wins:
                epilogue(w, paggs[w])

    nc.compile()
    return nc


def kernel(x, edge_index, W, b, alpha):
    from concourse.bass_utils import run_bass_kernel_spmd

    t0 = time.time()
    x = np.ascontiguousarray(np.asarray(x, dtype=np.float32))
    W = np.asarray(W, dtype=np.float32)
    b = np.asarray(b, dtype=np.float32)
    alpha = np.asarray(alpha, dtype=np.float32)

    per_core, T, tile_start, NT, xs = _plan(edge_index, x, W, alpha)
    fast_path = bool(np.all(b == 0.0) and np.all(alpha > 0.0))
    uniform_alpha = bool(np.all(b == 0.0) and np.all(alpha == alpha[0])
                         and alpha[0] >= 0.0)
    alpha0 = float(alpha[0])

    wt = np.ascontiguousarray(W.T.astype(np.float16))                     # [F, H]
    w1t = np.ascontiguousarray((alpha[:, None] * W).T.astype(np.float16))  # [F, H]

    t1 = time.time()
    nc = _build_program(T, tile_start, NT, fast_path, uniform_alpha, alpha0)
    t2 = time.time()

    in_maps = []
    for k in range(NC_CORES):
        d = per_core[k]
        m = {
            "xs": xs, "w_t": wt, "w1_t": w1t,
            "xloc": d["xloc"],
            "idx0": d[("idx", 0)], "idx1": d[("idx", 1)],
            "dlc0": d[("dlc", 0)], "dlc1": d[("dlc", 1)],
            "dv": d["dv"], "dvn": d["dvn"],
        }
        if not fast_path:
            m["alpha_row"] = alpha.reshape(1, H).astype(np.float32)
            m["b_row"] = b.reshape(1, H).astype(np.float32)
        in_maps.append(m)

    trace = bool(int(os.environ.get("GCN_BASS_TRACE", "0")))
    res = run_bass_kernel_spmd(nc, in_maps, core_ids=list(range(NC_CORES)),
                               trace=trace)
    t3 = time.time()

    outs = np.stack([res.results[k]["out"][:ND].astype(np.float32)
                     for k in range(NC_CORES)])  # [8, 6250, H]
    out_full = outs.transpose(1, 0, 2).reshape(N, H)
    t4 = time.time()

    last_run_info.update({
        "exec_time_ns": res.exec_time_ns,
        "plan_s": t1 - t0, "build_s": t2 - t1, "run_s": t3 - t2,
        "unshard_s": t4 - t3, "fast_path": fast_path,
        "uniform_alpha": uniform_alpha,
        "NT": NT, "trace": trace,
    })
    return out_full


# revision 11
# speedup vs baseline: 1.0732x; 1.0732x over previous
"""GCN message-passing kernel (nn_Encoder_21646635172361) for 8 Trainium2 cores.

Math (reference):
    h   = x @ W.T                     [N,H]
    A~  = adjacency + self loops, symmetric-normalized: norm(r,c) = dinv[r]*dinv[c]
    out = PReLU(A~ @ h + b, alpha)

Key algebraic restructure: aggregation commutes with the linear transform,
    A~ @ (x W.T) = (A~ @ x) W.T
so we aggregate F=128-wide rows (4x less gather traffic than H=512).
    agg[c] = dinv[c] * ( sum_{r->c} dinv[r]*x[r] + dinv[c]*x[c] )
Host prescales xs = dinv*x (fp16), so per-edge messages are plain rows of xs
and the scatter-add becomes binary one-hot matmuls on the TensorEngine.
Self-loops are NOT gathered: their xs rows are shipped per-core in local
order (xloc) and folded into each window's chain via one identity mask;
the epilogue's dinv[dst] completes dinv^2.

Distribution: destination nodes sharded round-robin (dst % 8) across the 8
cores; xs replicated; each core gathers the source rows for its own edges
(DistGNN-style edge partition, no collectives needed).

Performance notes (v4, measured on HW):
 - dma_gather desc-gen runs on the Q7 core pair (2q, 2q+1) of its SWDGE
   queue q: ~8.6us per 1024-idx call per queue, but the 4 queues work
   CONCURRENTLY.  Round-robin chunks across all 4 queues -> ~2.2us/call
   effective (~2.1 ns/idx), 4x the single-queue baseline.  All gather
   chunks stay SBUF-resident so the stream never stalls on recycling.
 - TRN2 PE accumulation chains into a single PSUM bank stall ~800ns/matmul
   on the bank read-modify-write; interleaving 4 windows' chains across 4
   PSUM banks runs at ~136ns per matmul.  WIN=128 keeps the one-hot rhs
   small (PE 256cyc/tile) and the epilogue one block per window.
 - One-hots are built one DVE is_equal per 8-tile chunk (materialized
   repeating iota, fp16 dlc broadcast via stride-0 AP) instead of one op
   per tile.
 - Uniform alpha (the PReLU default init) collapses the epilogue to a
   single matmul + one scalar Prelu activation per window.
 - Output is written fp16 (half the HBM write traffic); host converts to
   fp32.  absmax-rel error stays ~5e-4, well under the 2e-2 gate.
"""

import os
import time
from contextlib import ExitStack

import numpy as np

N, F, H = 50000, 128, 512
NC_CORES = 8
ND = N // NC_CORES            # 6250 local dst nodes per core
WIN = 256                     # dst window width
NW = (ND + WIN - 1) // WIN    # 25 windows
NDP = NW * WIN                # 6400 padded local dst rows
SPLIT = 32768                 # int16 gather index split point
CH = 1024                     # gather chunk size (edges per dma_gather; >1024 fails on HW)
TILE = 128                    # edges per PE tile
CHB = CH // TILE              # tiles per chunk (8)
GRP = 4                       # windows interleaved per PSUM-bank group

# Results of the last kernel() call (for test.py introspection)
last_run_info = {}


def _plan(edge_index, x, W, alpha):
    """Host-side graph partition + input prep. Returns per-core arrays +
    shared tile structure (uniform across cores, required for SPMD)."""
    src = np.asarray(edge_index[0], dtype=np.int64)
    dst = np.asarray(edge_index[1], dtype=np.int64)

    # degrees include self-loops (reference adds them)
    deg = np.bincount(dst, minlength=N) + 1
    dinv = (1.0 / np.sqrt(deg.astype(np.float64))).astype(np.float32)

    # xs rows serve both edge messages (epilogue adds dinv[dst]) and
    # self-loops (same epilogue factor completes dinv[d]^2).
    xs = (dinv[:, None] * x).astype(np.float16)           # [N, F]

    core = (dst % NC_CORES).astype(np.int64)
    loc = dst // NC_CORES
    win = loc // WIN
    dloc = (loc % WIN).astype(np.float32)
    low = src < SPLIT

    # group edges per (core, window, stream)
    grp = {}
    for k in range(NC_CORES):
        mk = core == k
        s_k, w_k, dl_k, lo_k = src[mk], win[mk], dloc[mk], low[mk]
        for w in range(NW):
            mw = w_k == w
            s_w, dl_w, lo_w = s_k[mw], dl_k[mw], lo_k[mw]
            grp[(k, w, 0)] = (s_w[lo_w], dl_w[lo_w])
            grp[(k, w, 1)] = (s_w[~lo_w] - SPLIT, dl_w[~lo_w])

    # uniform tile counts across cores
    T = np.zeros((2, NW), dtype=np.int64)
    for st in range(2):
        for w in range(NW):
            cnt = max(len(grp[(k, w, st)][0]) for k in range(NC_CORES))
            T[st, w] = (cnt + TILE - 1) // TILE
    tile_start = np.zeros((2, NW), dtype=np.int64)
    tile_start[0, 1:] = np.cumsum(T[0])[:-1]
    tile_start[1, 1:] = np.cumsum(T[1])[:-1]
    NT = [int(T[0].sum()), int(T[1].sum())]

    per_core = []
    for k in range(NC_CORES):
        core_dat = {}
        for st in range(2):
            nt = NT[st]
            idx = np.zeros(nt * TILE, dtype=np.int16)      # pad -> row 0 (harmless)
            dlc = np.full(nt * TILE, -1.0, dtype=np.float16)  # pad -> no one-hot match
            for w in range(NW):
                s_w, dl_w = grp[(k, w, st)]
                o = tile_start[st, w] * TILE
                n = len(s_w)
                idx[o:o + n] = s_w.astype(np.int16)
                dlc[o:o + n] = dl_w.astype(np.float16)
            # wrapped int16 index layout: [p, j] = idx[j*16 + p%16], replicated
            wrapped = idx.reshape(-1, 16).T            # [16, nt*8]
            wrapped = np.tile(wrapped, (8, 1)).copy()  # [128, nt*8]
            core_dat[("idx", st)] = wrapped
            core_dat[("dlc", st)] = dlc.reshape(nt, TILE).T.copy()  # [128, nt] fp16
        # local dst rows (xloc) and dinv tables in window-local order
        g = np.arange(NDP, dtype=np.int64) * NC_CORES + k
        valid = np.arange(NDP) < ND
        gc = np.minimum(g, N - 1)
        xl = np.where(valid[:, None], xs[gc], np.float16(0.0))      # [NDP, F]
        core_dat["xloc"] = np.ascontiguousarray(xl)
        dv = np.where(valid, dinv[gc], 0.0).astype(np.float32)
        core_dat["dv"] = dv.reshape(2 * NW, TILE).T.copy()          # [128, 2NW]
        core_dat["dvn"] = (-core_dat["dv"]).copy()
        per_core.append(core_dat)

    return per_core, T, tile_start, NT, xs


def _build_program(T, tile_start, NT, fast_path, uniform_alpha, alpha0):
    import concourse.mybir as mybir
    import concourse.tile as tile
    from concourse import bacc

    f32 = mybir.dt.float32
    fp16 = mybir.dt.float16
    i16 = mybir.dt.int16
    Alu = mybir.AluOpType
    Act = mybir.ActivationFunctionType

    nc = bacc.Bacc("TRN2", target_bir_lowering=False, debug=False,
                   num_devices=NC_CORES, num_swdge_queues=4)

    xs_d = nc.dram_tensor("xs", [N, F], fp16, kind="ExternalInput").ap()
    xloc_d = nc.dram_tensor("xloc", [NDP, F], fp16, kind="ExternalInput").ap()
    wt_d = nc.dram_tensor("w_t", [F, H], fp16, kind="ExternalInput").ap()
    w1t_d = nc.dram_tensor("w1_t", [F, H], fp16, kind="ExternalInput").ap()
    idx_d = [nc.dram_tensor(f"idx{st}", [128, NT[st] * 8], i16,
                            kind="ExternalInput").ap() for st in range(2)]
    dlc_d = [nc.dram_tensor(f"dlc{st}", [128, NT[st]], fp16,
                            kind="ExternalInput").ap() for st in range(2)]
    dv_d = nc.dram_tensor("dv", [128, 2 * NW], f32, kind="ExternalInput").ap()
    dvn_d = nc.dram_tensor("dvn", [128, 2 * NW], f32, kind="ExternalInput").ap()
    iot_d = nc.dram_tensor("iota_rep", [128, CHB * WIN], fp16,
                           kind="ExternalInput").ap()
    sfo_d = nc.dram_tensor("selfoh", [128, 2 * WIN], fp16,
                           kind="ExternalInput").ap()
    if not fast_path:
        arow_d = nc.dram_tensor("alpha_row", [1, H], f32, kind="ExternalInput").ap()
        brow_d = nc.dram_tensor("b_row", [1, H], f32, kind="ExternalInput").ap()
    out_d = nc.dram_tensor("out", [NDP, H], fp16, kind="ExternalOutput").ap()

    xs_lo = xs_d[0:SPLIT, :]
    xs_hi = xs_d[SPLIT:N, :]
    x_in = [xs_lo, xs_hi]

    n_chunks = [(NT[st] * TILE + CH - 1) // CH for st in range(2)]
    n_chunks_tot = n_chunks[0] + n_chunks[1]

    with tile.TileContext(nc) as tc, ExitStack() as ctx:
        cpool = ctx.enter_context(tc.tile_pool(name="const", bufs=1))
        gxpool = ctx.enter_context(tc.tile_pool(name="gx", bufs=min(32, n_chunks_tot)))
        xlpool = ctx.enter_context(tc.tile_pool(name="xl", bufs=10))
        ohpool = ctx.enter_context(tc.tile_pool(name="oh", bufs=16))
        aggpool = ctx.enter_context(tc.tile_pool(name="aggs", bufs=2))
        eppool = ctx.enter_context(tc.tile_pool(name="ep", bufs=2))
        ps_agg = ctx.enter_context(tc.tile_pool(name="ps_agg", bufs=1, space="PSUM"))
        ps_out = ctx.enter_context(tc.tile_pool(name="ps_out", bufs=2, space="PSUM"))

        # ---- one-time loads (idx tables first: the gather stream waits on them) ----
        idx_sb, dlc_sb = [], []
        for st in range(2):
            t = cpool.tile([128, NT[st] * 8], i16, tag=f"idx{st}")
            nc.sync.dma_start(t[:], idx_d[st])
            idx_sb.append(t)
        for st in range(2):
            t = cpool.tile([128, NT[st]], fp16, tag=f"dlc{st}")
            nc.sync.dma_start(t[:], dlc_d[st])
            dlc_sb.append(t)
        dv_sb = cpool.tile([128, 2 * NW], f32)
        nc.sync.dma_start(dv_sb[:], dv_d)
        dvn_sb = cpool.tile([128, 2 * NW], f32)
        nc.sync.dma_start(dvn_sb[:], dvn_d)
        wt_sb = cpool.tile([F, H], fp16)
        nc.sync.dma_start(wt_sb[:], wt_d)
        if fast_path and not uniform_alpha:
            w1t_sb = cpool.tile([F, H], fp16)
            nc.sync.dma_start(w1t_sb[:], w1t_d)

        # host-shipped repeating iota + self-loop masks: keeps GpSimd free
        # to run ONLY dma_gathers (no library switch before the first one)
        iota_flat = cpool.tile([128, CHB * WIN], fp16)
        nc.sync.dma_start(iota_flat[:], iot_d)
        iota_rep = iota_flat.rearrange("p (b w) -> p b w", b=CHB)
        sfo_sb = cpool.tile([128, 2 * WIN], fp16)
        nc.sync.dma_start(sfo_sb[:], sfo_d)
        selfoh = [sfo_sb[:, 0:WIN], sfo_sb[:, WIN:2 * WIN]]

        if not fast_path:
            ones_sb = cpool.tile([1, 128], f32)
            nc.vector.memset(ones_sb[:], 1.0)
            arow_sb = cpool.tile([1, H], f32)
            nc.sync.dma_start(arow_sb[:], arow_d)
            brow_sb = cpool.tile([1, H], f32)
            nc.sync.dma_start(brow_sb[:], brow_d)
            arep_ps = ps_out.tile([128, H], f32, tag="brd")
            nc.tensor.matmul(arep_ps[:], lhsT=ones_sb[:], rhs=arow_sb[:],
                             start=True, stop=True)
            arep_sb = cpool.tile([128, H], f32)
            nc.scalar.copy(arep_sb[:], arep_ps[:])
            brep_ps = ps_out.tile([128, H], f32, tag="brd")
            nc.tensor.matmul(brep_ps[:], lhsT=ones_sb[:], rhs=brow_sb[:],
                             start=True, stop=True)
            brep_sb = cpool.tile([128, H], f32)
            nc.scalar.copy(brep_sb[:], brep_ps[:])

        # ---- main loop ----
        gx_tiles = [[None] * n_chunks[0], [None] * n_chunks[1]]
        oh_tiles = [[None] * n_chunks[0], [None] * n_chunks[1]]
        q_counter = [0]

        def chunk_tile(st, c):
            if gx_tiles[st][c] is None:
                num = min(CH, NT[st] * TILE - c * CH)
                nblk = num // TILE
                gx = gxpool.tile([128, CHB, TILE], fp16, tag="gx")
                nc.gpsimd.dma_gather(
                    out_ap=gx[:, 0:nblk, :],
                    in_ap=x_in[st],
                    idxs_ap=idx_sb[st][:, c * (CH // 16): c * (CH // 16) + num // 16],
                    num_idxs=num,
                    num_idxs_reg=num,
                    elem_size=F,
                    queue_num=q_counter[0] % 4,
                )
                q_counter[0] += 1
                gx_tiles[st][c] = gx
            return gx_tiles[st][c]

        def oh_chunk_tile(st, c):
            # one is_equal per chunk: materialized repeating iota vs the dlc
            # column broadcast over the WIN positions.
            if oh_tiles[st][c] is None:
                nblk = min(CHB, NT[st] - c * CHB)
                oh = ohpool.tile([128, CHB, WIN], fp16, tag="oh")
                dlc_b = (dlc_sb[st][:, c * CHB: c * CHB + nblk]
                         .unsqueeze(2).to_broadcast([128, nblk, WIN]))
                nc.vector.tensor_tensor(oh[:, 0:nblk, :],
                                        iota_rep[:, 0:nblk, :], dlc_b,
                                        op=Alu.is_equal)
                oh_tiles[st][c] = oh
            return oh_tiles[st][c]

        def epilogue(w, pagg):
            agg_sb = aggpool.tile([128, WIN], fp16, tag="aggs")
            nc.scalar.copy(agg_sb[:], pagg[:])
            for h2 in range(2):
                _epilogue_half(w, h2, agg_sb)

        def _epilogue_half(w, h2, agg_sb):
            hw = 2 * w + h2
            lhs = agg_sb[:, h2 * 128:(h2 + 1) * 128]
            dv_col = dv_sb[:, hw:hw + 1]
            ps0 = ps_out.tile([128, H], f32, tag="ps0")
            nc.tensor.matmul(ps0[:], lhsT=lhs, rhs=wt_sb[:],
                             start=True, stop=True)
            if uniform_alpha:
                # out = PReLU(dv*z0; alpha0): single activation, no z1
                outt = eppool.tile([128, H], fp16, tag="outt")
                nc.scalar.activation(outt[:], ps0[:], Act.Prelu,
                                     scale=dv_col, alpha=float(alpha0))
            elif fast_path:
                # out = relu(dv*z0) - relu(-dv*z1), z1 = agg @ (alpha W)^T
                ps1 = ps_out.tile([128, H], f32, tag="ps1")
                nc.tensor.matmul(ps1[:], lhsT=lhs, rhs=w1t_sb[:],
                                 start=True, stop=True)
                pos = eppool.tile([128, H], f32, tag="pos")
                nc.scalar.activation(pos[:], ps0[:], Act.Relu, scale=dv_col)
                neg = eppool.tile([128, H], f32, tag="neg")
                nc.scalar.activation(neg[:], ps1[:], Act.Relu,
                                     scale=dvn_sb[:, hw:hw + 1])
                outt = eppool.tile([128, H], fp16, tag="outt")
                nc.vector.tensor_tensor(outt[:], pos[:], neg[:],
                                        op=Alu.subtract)
            else:
                # general: v = dv*z0 + b; out = relu(v) + alpha*min(v,0)
                vb = eppool.tile([128, H], f32, tag="vb")
                nc.vector.tensor_scalar(vb[:], ps0[:], dv_col, None,
                                        op0=Alu.mult)
                vb2 = eppool.tile([128, H], f32, tag="vb2")
                nc.vector.tensor_tensor(vb2[:], vb[:], brep_sb[:],
                                        op=Alu.add)
                pos = eppool.tile([128, H], f32, tag="pos")
                nc.scalar.activation(pos[:], vb2[:], Act.Relu)
                neg = eppool.tile([128, H], f32, tag="neg")
                nc.vector.tensor_scalar(neg[:], vb2[:], 0.0, None,
                                        op0=Alu.min)
                nega = eppool.tile([128, H], f32, tag="nega")
                nc.vector.tensor_tensor(nega[:], neg[:], arep_sb[:],
                                        op=Alu.mult)
                outt = eppool.tile([128, H], fp16, tag="outt")
                nc.vector.tensor_tensor(outt[:], pos[:], nega[:],
                                        op=Alu.add)
            nc.sync.dma_start(out_d[hw * 128:(hw + 1) * 128, :], outt[:])

        # windows in groups of GRP; interleave matmul emission across the
        # group so consecutive PE instructions hit different PSUM banks.
        for g0 in range(0, NW, GRP):
            wins = list(range(g0, min(g0 + GRP, NW)))
            paggs = {}
            mm_count = {}
            n_mm = {}
            for j, w in enumerate(wins):
                paggs[w] = ps_agg.tile([128, WIN], f32, name=f"pagg{j}",
                                       tag=f"pagg{j}")
                mm_count[w] = 0
                n_mm[w] = 2 + int(T[0, w] + T[1, w])

            # self-loop matmuls first (start each window's chain)
            for h in range(2):
                for w in wins:
                    xl = xlpool.tile([128, F], fp16, tag="xl")
                    r0 = (2 * w + h) * 128
                    nc.sync.dma_start(xl[:], xloc_d[r0:r0 + 128, :])
                    nc.tensor.matmul(paggs[w][:], lhsT=xl[:], rhs=selfoh[h],
                                     start=(mm_count[w] == 0),
                                     stop=(mm_count[w] == n_mm[w] - 1))
                    mm_count[w] += 1

            # round-robin the gather tiles across the group's windows
            tl = {w: [(st, t) for st in range(2) for t in range(int(T[st, w]))]
                  for w in wins}
            pos_i = {w: 0 for w in wins}
            remaining = sum(len(v) for v in tl.values())
            while remaining:
                for w in wins:
                    if pos_i[w] >= len(tl[w]):
                        continue
                    st, t = tl[w][pos_i[w]]
                    pos_i[w] += 1
                    remaining -= 1
                    gt = int(tile_start[st, w]) + t
                    c, blk = divmod(gt, CHB)
                    gx = chunk_tile(st, c)
                    oh = oh_chunk_tile(st, c)
                    nc.tensor.matmul(
                        paggs[w][:],
                        lhsT=gx[:, blk:blk + 1, :],
                        rhs=oh[:, blk, :],
                        start=(mm_count[w] == 0),
                        stop=(mm_count[w] == n_mm[w] - 1),
                    )
                    mm_count[w] += 1

            for w in wins:
                epilogue(w, paggs[w])

    nc.compile()
    return nc


def kernel(x, edge_index, W, b, alpha):
    from concourse.bass_utils import run_bass_kernel_spmd

    t0 = time.time()
    x = np.ascontiguousarray(np.asarray(x, dtype=np.float32))
    W = np.asarray(W, dtype=np.float32)
    b = np.asarray(b, dtype=np.float32)
    alpha = np.asarray(alpha, dtype=np.float32)

    per_core, T, tile_start, NT, xs = _plan(edge_index, x, W, alpha)
    fast_path = bool(np.all(b == 0.0) and np.all(alpha > 0.0))
    uniform_alpha = bool(np.all(b == 0.0) and np.all(alpha == alpha[0])
                         and alpha[0] >= 0.0)
    alpha0 = float(alpha[0])

    wt = np.ascontiguousarray(W.T.astype(np.float16))                     # [F, H]
    w1t = np.ascontiguousarray((alpha[:, None] * W).T.astype(np.float16))  # [F, H]
    iota_rep = np.tile(np.arange(WIN, dtype=np.float16), (128, CHB))       # [128, CHB*WIN]
    iota_rep = np.ascontiguousarray(iota_rep)
    p = np.arange(128)
    sfo = np.concatenate([
        (np.arange(WIN)[None, :] == p[:, None]),
        (np.arange(WIN)[None, :] == (p + 128)[:, None])], axis=1)
    selfoh_h = np.ascontiguousarray(sfo.astype(np.float16))                # [128, 2*WIN]

    t1 = time.time()
    nc = _build_program(T, tile_start, NT, fast_path, uniform_alpha, alpha0)
    t2 = time.time()

    in_maps = []
    for k in range(NC_CORES):
        d = per_core[k]
        m = {
            "xs": xs, "w_t": wt, "w1_t": w1t,
            "xloc": d["xloc"],
            "idx0": d[("idx", 0)], "idx1": d[("idx", 1)],
            "dlc0": d[("dlc", 0)], "dlc1": d[("dlc", 1)],
            "dv": d["dv"], "dvn": d["dvn"],
            "iota_rep": iota_rep, "selfoh": selfoh_h,
        }
        if not fast_path:
            m["alpha_row"] = alpha.reshape(1, H).astype(np.float32)
            m["b_row"] = b.reshape(1, H).astype(np.float32)
        in_maps.append(m)

    trace = bool(int(os.environ.get("GCN_BASS_TRACE", "0")))
    res = run_bass_kernel_spmd(nc, in_maps, core_ids=list(range(NC_CORES)),
                               trace=trace)
    t3 = time.time()

    outs = np.stack([res.results[k]["out"][:ND].astype(np.float32)
                     for k in range(NC_CORES)])  # [8, 6250, H]
    out_full = outs.transpose(1, 0, 2).reshape(N, H)
    t4 = time.time()

    last_run_info.update({
        "exec_time_ns": res.exec_time_ns,
        "plan_s": t1 - t0, "build_s": t2 - t1, "run_s": t3 - t2,
        "unshard_s": t4 - t3, "fast_path": fast_path,
        "uniform_alpha": uniform_alpha,
        "NT": NT, "trace": trace,
    })
    return out_full


# revision 14
# speedup vs baseline: 1.1185x; 1.0422x over previous
"""GCN message-passing kernel (nn_Encoder_21646635172361) for 8 Trainium2 cores.

Math (reference):
    h   = x @ W.T                     [N,H]
    A~  = adjacency + self loops, symmetric-normalized: norm(r,c) = dinv[r]*dinv[c]
    out = PReLU(A~ @ h + b, alpha)

Key algebraic restructure: aggregation commutes with the linear transform,
    A~ @ (x W.T) = (A~ @ x) W.T
so we aggregate F=128-wide rows (4x less gather traffic than H=512).
    agg[c] = dinv[c] * ( sum_{r->c} dinv[r]*x[r] + dinv[c]*x[c] )
Host prescales xs = dinv*x (fp16), so per-edge messages are plain rows of xs
and the scatter-add becomes binary one-hot matmuls on the TensorEngine.
Self-loops are NOT gathered: their xs rows are shipped per-core in local
order (xloc) and folded in via one identity mask; the epilogue's dinv[dst]
completes dinv^2.

Distribution: destination nodes sharded round-robin (dst % 8) across the 8
cores; xs replicated; each core gathers the source rows for its own edges
(DistGNN-style edge partition, no collectives needed).

Performance notes (v6, measured on HW):
 - dma_gather desc-gen runs on the Q7 core pair (2q, 2q+1) of its SWDGE
   queue q: ~8.6us per 1024-idx call per queue, but the 4 queues work
   CONCURRENTLY.  Round-robin chunks across all 4 queues -> ~2.2us/call
   effective (~2.1 ns/idx), 4x the single-queue baseline.
 - SINGLE index stream: int16 gather indices are sign-extended by the Q7
   desc-gen, so with the HBM base at row 32768 the whole 50000-row table
   is addressable as rel = src - 32768 in [-32768, 17232).  (Verified on
   HW: mid-array negative indices gather the correct rows.  Only TRAILING
   negatives are dropped by the ucode, so tiles sort slots ascending and
   pads use idx 0.)  This halves the per-(window,stream) tile-rounding
   padding of the old split-stream layout.
 - TRN2 PE accumulation chains into one PSUM bank stall ~800ns/matmul on
   the bank read-modify-write; interleaving windows' chains across the 4
   PSUM agg banks runs at ~136ns/matmul.  WIN=128 keeps the one-hot rhs
   at 128 columns (256 PE cycles/tile) and 4 windows pack per 2KB bank.
 - One-hots are built one DVE is_equal per 8-tile chunk (~0.74us) from a
   host-shipped repeating iota; DVE is_equal runs ~1 elem/cycle, so the
   rhs width (=WIN) directly scales the one-hot cost.
 - Uniform alpha (the PReLU default init) collapses the epilogue to one
   matmul + one scalar Prelu activation per window.
 - A dummy 128-idx gather issues first so the ~6us GpSimd IRAM library
   load overlaps the index-table DMA instead of serializing after it.
 - Output is written fp16 (half the HBM write traffic); host converts to
   fp32.  absmax-rel error stays ~5e-4, well under the 2e-2 gate.
"""

import os
import time
from contextlib import ExitStack

import numpy as np

N, F, H = 50000, 128, 512
NC_CORES = 8
ND = N // NC_CORES            # 6250 local dst nodes per core
WIN = 128                     # dst window width (one output block)
NW = (ND + WIN - 1) // WIN    # 49 windows
NDP = NW * WIN                # 6272 padded local dst rows
BASE = 32768                  # gather base row (idx = src - BASE, int16)
CH = 1024                     # gather chunk size (edges per dma_gather; >1024 fails on HW)
TILE = 128                    # edges per PE tile
CHB = CH // TILE              # tiles per chunk (8)
WPB = 4                       # windows per PSUM bank ([128,512] f32)
NBANK = 4                     # agg banks interleaved
GRP = WPB * NBANK             # windows per emission group (16)

# Results of the last kernel() call (for test.py introspection)
last_run_info = {}


def _plan(edge_index, x, W, alpha):
    """Host-side graph partition + input prep. Returns per-core arrays +
    shared tile structure (uniform across cores, required for SPMD)."""
    src = np.asarray(edge_index[0], dtype=np.int64)
    dst = np.asarray(edge_index[1], dtype=np.int64)

    # degrees include self-loops (reference adds them)
    deg = np.bincount(dst, minlength=N) + 1
    dinv = (1.0 / np.sqrt(deg.astype(np.float64))).astype(np.float32)

    # xs rows serve both edge messages (epilogue adds dinv[dst]) and
    # self-loops (same epilogue factor completes dinv[d]^2).
    xs = (dinv[:, None] * x).astype(np.float16)           # [N, F]

    core = (dst % NC_CORES).astype(np.int64)
    loc = dst // NC_CORES
    win = loc // WIN
    dloc = (loc % WIN).astype(np.float32)
    rel = (src - BASE).astype(np.int64)                   # [-32768, 17232)

    # group edges per (core, window)
    grp = {}
    for k in range(NC_CORES):
        mk = core == k
        r_k, w_k, dl_k = rel[mk], win[mk], dloc[mk]
        for w in range(NW):
            mw = w_k == w
            grp[(k, w)] = (r_k[mw], dl_k[mw])

    # uniform tile counts across cores
    T = np.zeros(NW, dtype=np.int64)
    for w in range(NW):
        cnt = max(len(grp[(k, w)][0]) for k in range(NC_CORES))
        T[w] = (cnt + TILE - 1) // TILE
    tile_start = np.zeros(NW, dtype=np.int64)
    tile_start[1:] = np.cumsum(T)[:-1]
    NT = int(T.sum())

    per_core = []
    for k in range(NC_CORES):
        core_dat = {}
        idx = np.zeros(NT * TILE, dtype=np.int16)      # pad -> row BASE (harmless)
        dlc = np.full(NT * TILE, -1.0, dtype=np.float16)  # pad -> no one-hot match
        for w in range(NW):
            r_w, dl_w = grp[(k, w)]
            n = len(r_w)
            o = tile_start[w] * TILE
            # per tile: slots sorted by rel ascending (trailing-negative-drop
            # guard: real negatives never end a call; pads are idx 0 >= 0)
            for t in range((n + TILE - 1) // TILE):
                a, b = t * TILE, min((t + 1) * TILE, n)
                order = np.argsort(r_w[a:b], kind="stable")
                m = b - a
                idx[o + t * TILE: o + t * TILE + m] = r_w[a:b][order].astype(np.int16)
                dlc[o + t * TILE: o + t * TILE + m] = dl_w[a:b][order].astype(np.float16)
        # trailing-drop guard: the last slot of each 1024-chunk must be >= 0
        nchunks = (NT * TILE + CH - 1) // CH
        for c in range(nchunks):
            last = min((c + 1) * CH, NT * TILE) - 1
            assert idx[last] >= 0, "all-negative chunk tail (astronomically unlikely)"
        wrapped = idx.reshape(-1, 16).T            # [16, NT*8]
        wrapped = np.tile(wrapped, (8, 1)).copy()  # [128, NT*8]
        core_dat["idx"] = wrapped
        core_dat["dlc"] = dlc.reshape(NT, TILE).T.copy()  # [128, NT] fp16
        # local dst rows (xloc) and dinv tables in window-local order
        g = np.arange(NDP, dtype=np.int64) * NC_CORES + k
        valid = np.arange(NDP) < ND
        gc = np.minimum(g, N - 1)
        xl = np.where(valid[:, None], xs[gc], np.float16(0.0))      # [NDP, F]
        core_dat["xloc"] = np.ascontiguousarray(xl)
        dv = np.where(valid, dinv[gc], 0.0).astype(np.float32)
        core_dat["dv"] = dv.reshape(NW, TILE).T.copy()              # [128, NW]
        core_dat["dvn"] = (-core_dat["dv"]).copy()
        per_core.append(core_dat)

    return per_core, T, tile_start, NT, xs


def _build_program(T, tile_start, NT, fast_path, uniform_alpha, alpha0):
    import concourse.mybir as mybir
    import concourse.tile as tile
    from concourse import bacc

    f32 = mybir.dt.float32
    fp16 = mybir.dt.float16
    i16 = mybir.dt.int16
    Alu = mybir.AluOpType
    Act = mybir.ActivationFunctionType

    nc = bacc.Bacc("TRN2", target_bir_lowering=False, debug=False,
                   num_devices=NC_CORES, num_swdge_queues=4)

    xs_d = nc.dram_tensor("xs", [N, F], fp16, kind="ExternalInput").ap()
    xloc_d = nc.dram_tensor("xloc", [NDP, F], fp16, kind="ExternalInput").ap()
    wt_d = nc.dram_tensor("w_t", [F, H], fp16, kind="ExternalInput").ap()
    w1t_d = nc.dram_tensor("w1_t", [F, H], fp16, kind="ExternalInput").ap()
    idx_d = nc.dram_tensor("idx", [128, NT * 8], i16, kind="ExternalInput").ap()
    dlc_d = nc.dram_tensor("dlc", [128, NT], fp16, kind="ExternalInput").ap()
    dv_d = nc.dram_tensor("dv", [128, NW], f32, kind="ExternalInput").ap()
    dvn_d = nc.dram_tensor("dvn", [128, NW], f32, kind="ExternalInput").ap()
    iot_d = nc.dram_tensor("iota_rep", [128, CHB * WIN], fp16,
                           kind="ExternalInput").ap()
    sfo_d = nc.dram_tensor("selfoh", [128, WIN], fp16,
                           kind="ExternalInput").ap()
    if not fast_path:
        arow_d = nc.dram_tensor("alpha_row", [1, H], f32, kind="ExternalInput").ap()
        brow_d = nc.dram_tensor("b_row", [1, H], f32, kind="ExternalInput").ap()
    out_d = nc.dram_tensor("out", [NDP, H], fp16, kind="ExternalOutput").ap()

    x_base = xs_d[BASE:BASE + 1, :]

    n_chunks = (NT * TILE + CH - 1) // CH

    with tile.TileContext(nc) as tc, ExitStack() as ctx:
        cpool = ctx.enter_context(tc.tile_pool(name="const", bufs=1))
        gxpool = ctx.enter_context(tc.tile_pool(name="gx", bufs=min(36, n_chunks)))
        xlpool = ctx.enter_context(tc.tile_pool(name="xl", bufs=16))
        ohpool = ctx.enter_context(tc.tile_pool(name="oh", bufs=24))
        aggpool = ctx.enter_context(tc.tile_pool(name="aggs", bufs=2))
        eppool = ctx.enter_context(tc.tile_pool(name="ep", bufs=2))
        ps_agg = ctx.enter_context(tc.tile_pool(name="ps_agg", bufs=1, space="PSUM"))
        ps_out = ctx.enter_context(tc.tile_pool(name="ps_out", bufs=2, space="PSUM"))

        # dummy gather: triggers the ~6us GpSimd IRAM library load NOW so it
        # overlaps the idx-table DMA (the real gathers wait on that DMA).
        dum_idx = cpool.tile([128, 8], i16)
        nc.gpsimd.memset(dum_idx[:], 0)
        dum_gx = cpool.tile([128, 1, F], fp16)
        nc.gpsimd.dma_gather(out_ap=dum_gx[:], in_ap=x_base,
                             idxs_ap=dum_idx[:], num_idxs=128,
                             num_idxs_reg=128, elem_size=F, queue_num=0)

        # ---- one-time loads (idx table first: the gather stream waits on it) ----
        idx_sb = cpool.tile([128, NT * 8], i16)
        nc.sync.dma_start(idx_sb[:], idx_d)
        dlc_sb = cpool.tile([128, NT], fp16)
        nc.sync.dma_start(dlc_sb[:], dlc_d)
        iota_flat = cpool.tile([128, CHB * WIN], fp16)
        nc.sync.dma_start(iota_flat[:], iot_d)
        iota_rep = iota_flat.rearrange("p (b w) -> p b w", b=CHB)
        sfo_sb = cpool.tile([128, WIN], fp16)
        nc.sync.dma_start(sfo_sb[:], sfo_d)
        dv_sb = cpool.tile([128, NW], f32)
        nc.sync.dma_start(dv_sb[:], dv_d)
        dvn_sb = cpool.tile([128, NW], f32)
        nc.sync.dma_start(dvn_sb[:], dvn_d)
        wt_sb = cpool.tile([F, H], fp16)
        nc.sync.dma_start(wt_sb[:], wt_d)
        if fast_path and not uniform_alpha:
            w1t_sb = cpool.tile([F, H], fp16)
            nc.sync.dma_start(w1t_sb[:], w1t_d)

        if not fast_path:
            ones_sb = cpool.tile([1, 128], f32)
            nc.vector.memset(ones_sb[:], 1.0)
            arow_sb = cpool.tile([1, H], f32)
            nc.sync.dma_start(arow_sb[:], arow_d)
            brow_sb = cpool.tile([1, H], f32)
            nc.sync.dma_start(brow_sb[:], brow_d)
            arep_ps = ps_out.tile([128, H], f32, tag="ps0")
            nc.tensor.matmul(arep_ps[:], lhsT=ones_sb[:], rhs=arow_sb[:],
                             start=True, stop=True)
            arep_sb = cpool.tile([128, H], f32)
            nc.scalar.copy(arep_sb[:], arep_ps[:])
            brep_ps = ps_out.tile([128, H], f32, tag="ps0")
            nc.tensor.matmul(brep_ps[:], lhsT=ones_sb[:], rhs=brow_sb[:],
                             start=True, stop=True)
            brep_sb = cpool.tile([128, H], f32)
            nc.scalar.copy(brep_sb[:], brep_ps[:])

        # ---- main loop ----
        gx_tiles = [None] * n_chunks
        oh_tiles = [None] * n_chunks
        q_counter = [1]  # queue 0 used by the dummy gather

        def chunk_tile(c):
            if gx_tiles[c] is None:
                num = min(CH, NT * TILE - c * CH)
                nblk = num // TILE
                gx = gxpool.tile([128, CHB, TILE], fp16, tag="gx")
                nc.gpsimd.dma_gather(
                    out_ap=gx[:, 0:nblk, :],
                    in_ap=x_base,
                    idxs_ap=idx_sb[:, c * (CH // 16): c * (CH // 16) + num // 16],
                    num_idxs=num,
                    num_idxs_reg=num,
                    elem_size=F,
                    queue_num=q_counter[0] % 4,
                )
                q_counter[0] += 1
                gx_tiles[c] = gx
            return gx_tiles[c]

        def oh_chunk_tile(c):
            if oh_tiles[c] is None:
                nblk = min(CHB, NT - c * CHB)
                oh = ohpool.tile([128, CHB, WIN], fp16, tag="oh")
                dlc_b = (dlc_sb[:, c * CHB: c * CHB + nblk]
                         .unsqueeze(2).to_broadcast([128, nblk, WIN]))
                nc.vector.tensor_tensor(oh[:, 0:nblk, :],
                                        iota_rep[:, 0:nblk, :], dlc_b,
                                        op=Alu.is_equal)
                oh_tiles[c] = oh
            return oh_tiles[c]

        def epilogue_win(w, agg_slice):
            dv_col = dv_sb[:, w:w + 1]
            ps0 = ps_out.tile([128, H], f32, tag="ps0")
            nc.tensor.matmul(ps0[:], lhsT=agg_slice, rhs=wt_sb[:],
                             start=True, stop=True)
            if uniform_alpha:
                # out = PReLU(dv*z0; alpha0): single activation, no z1
                outt = eppool.tile([128, H], fp16, tag="outt")
                nc.scalar.activation(outt[:], ps0[:], Act.Prelu,
                                     scale=dv_col, alpha=float(alpha0))
            elif fast_path:
                # out = relu(dv*z0) - relu(-dv*z1), z1 = agg @ (alpha W)^T
                ps1 = ps_out.tile([128, H], f32, tag="ps1")
                nc.tensor.matmul(ps1[:], lhsT=agg_slice, rhs=w1t_sb[:],
                                 start=True, stop=True)
                pos = eppool.tile([128, H], f32, tag="pos")
                nc.scalar.activation(pos[:], ps0[:], Act.Relu, scale=dv_col)
                neg = eppool.tile([128, H], f32, tag="neg")
                nc.scalar.activation(neg[:], ps1[:], Act.Relu,
                                     scale=dvn_sb[:, w:w + 1])
                outt = eppool.tile([128, H], fp16, tag="outt")
                nc.vector.tensor_tensor(outt[:], pos[:], neg[:],
                                        op=Alu.subtract)
            else:
                # general: v = dv*z0 + b; out = relu(v) + alpha*min(v,0)
                vb = eppool.tile([128, H], f32, tag="vb")
                nc.vector.tensor_scalar(vb[:], ps0[:], dv_col, None,
                                        op0=Alu.mult)
                vb2 = eppool.tile([128, H], f32, tag="vb2")
                nc.vector.tensor_tensor(vb2[:], vb[:], brep_sb[:],
                                        op=Alu.add)
                pos = eppool.tile([128, H], f32, tag="pos")
                nc.scalar.activation(pos[:], vb2[:], Act.Relu)
                neg = eppool.tile([128, H], f32, tag="neg")
                nc.vector.tensor_scalar(neg[:], vb2[:], 0.0, None,
                                        op0=Alu.min)
                nega = eppool.tile([128, H], f32, tag="nega")
                nc.vector.tensor_tensor(nega[:], neg[:], arep_sb[:],
                                        op=Alu.mult)
                outt = eppool.tile([128, H], fp16, tag="outt")
                nc.vector.tensor_tensor(outt[:], pos[:], nega[:],
                                        op=Alu.add)
            nc.sync.dma_start(out_d[w * WIN:(w + 1) * WIN, :], outt[:])

        # windows in groups of GRP=16; 4 windows pack into each of 4 PSUM
        # banks; emission rotates banks instruction-by-instruction so the
        # PSUM read-modify-write latency is hidden.
        for g0 in range(0, NW, GRP):
            wins = list(range(g0, min(g0 + GRP, NW)))
            banks = [ps_agg.tile([128, WPB * WIN], f32, name=f"pagg{b}",
                                 tag=f"pagg{b}")
                     for b in range(NBANK)]
            slot = {}            # w -> (bank, sub)
            for i, w in enumerate(wins):
                slot[w] = (i % NBANK, i // NBANK)
            mm_count = {w: 0 for w in wins}
            n_mm = {w: 1 + int(T[w]) for w in wins}

            def pagg_slice(w):
                b, j = slot[w]
                return banks[b][:, j * WIN:(j + 1) * WIN]

            # bank-rotating window order
            worder = [wins[b + NBANK * j] for j in range(WPB)
                      for b in range(NBANK) if b + NBANK * j < len(wins)]

            # self-loop matmuls first.  start=True ZEROES THE WHOLE PSUM BANK
            # (measured on HW), so exactly one start per bank: the first
            # window's self matmul resets the bank, the other sub-windows
            # accumulate onto the zeroed regions.
            bank_started = set()
            for w in worder:
                xl = xlpool.tile([128, F], fp16, tag="xl")
                r0 = w * WIN
                nc.sync.dma_start(xl[:], xloc_d[r0:r0 + WIN, :])
                b = slot[w][0]
                nc.tensor.matmul(pagg_slice(w), lhsT=xl[:], rhs=sfo_sb[:],
                                 start=(b not in bank_started),
                                 stop=(mm_count[w] == n_mm[w] - 1),
                                 skip_group_check=True)
                bank_started.add(b)
                mm_count[w] += 1

            # round-robin the gather tiles across the group's windows
            pos_i = {w: 0 for w in wins}
            remaining = sum(int(T[w]) for w in wins)
            while remaining:
                for w in worder:
                    if pos_i[w] >= int(T[w]):
                        continue
                    t = pos_i[w]
                    pos_i[w] += 1
                    remaining -= 1
                    gt = int(tile_start[w]) + t
                    c, blk = divmod(gt, CHB)
                    gx = chunk_tile(c)
                    oh = oh_chunk_tile(c)
                    nc.tensor.matmul(
                        pagg_slice(w),
                        lhsT=gx[:, blk:blk + 1, :],
                        rhs=oh[:, blk, :],
                        start=False,
                        stop=(mm_count[w] == n_mm[w] - 1),
                        skip_group_check=True,
                    )
                    mm_count[w] += 1

            # per-bank agg copy (waits for its 4 windows' chains), then
            # per-window epilogue
            for b in range(NBANK):
                bw = [w for w in wins if slot[w][0] == b]
                if not bw:
                    continue
                nsub = max(slot[w][1] for w in bw) + 1
                agg_sb = aggpool.tile([128, WPB * WIN], fp16, tag="aggs")
                nc.scalar.copy(agg_sb[:, 0:nsub * WIN],
                               banks[b][:, 0:nsub * WIN])
                for w in bw:
                    j = slot[w][1]
                    epilogue_win(w, agg_sb[:, j * WIN:(j + 1) * WIN])

    nc.compile()
    return nc


def kernel(x, edge_index, W, b, alpha):
    from concourse.bass_utils import run_bass_kernel_spmd

    t0 = time.time()
    x = np.ascontiguousarray(np.asarray(x, dtype=np.float32))
    W = np.asarray(W, dtype=np.float32)
    b = np.asarray(b, dtype=np.float32)
    alpha = np.asarray(alpha, dtype=np.float32)

    per_core, T, tile_start, NT, xs = _plan(edge_index, x, W, alpha)
    fast_path = bool(np.all(b == 0.0) and np.all(alpha > 0.0))
    uniform_alpha = bool(np.all(b == 0.0) and np.all(alpha == alpha[0])
                         and alpha[0] >= 0.0)
    alpha0 = float(alpha[0])

    wt = np.ascontiguousarray(W.T.astype(np.float16))                     # [F, H]
    w1t = np.ascontiguousarray((alpha[:, None] * W).T.astype(np.float16))  # [F, H]
    iota_rep = np.tile(np.arange(WIN, dtype=np.float16), (128, CHB))       # [128, CHB*WIN]
    iota_rep = np.ascontiguousarray(iota_rep)
    p = np.arange(128)
    selfoh_h = np.ascontiguousarray(
        (np.arange(WIN)[None, :] == p[:, None]).astype(np.float16))        # [128, WIN]

    t1 = time.time()
    nc = _build_program(T, tile_start, NT, fast_path, uniform_alpha, alpha0)
    t2 = time.time()

    in_maps = []
    for k in range(NC_CORES):
        d = per_core[k]
        m = {
            "xs": xs, "w_t": wt, "w1_t": w1t,
            "xloc": d["xloc"],
            "idx": d["idx"], "dlc": d["dlc"],
            "dv": d["dv"], "dvn": d["dvn"],
            "iota_rep": iota_rep, "selfoh": selfoh_h,
        }
        if not fast_path:
            m["alpha_row"] = alpha.reshape(1, H).astype(np.float32)
            m["b_row"] = b.reshape(1, H).astype(np.float32)
        in_maps.append(m)

    trace = bool(int(os.environ.get("GCN_BASS_TRACE", "0")))
    res = run_bass_kernel_spmd(nc, in_maps, core_ids=list(range(NC_CORES)),
                               trace=trace)
    t3 = time.time()

    outs = np.stack([res.results[k]["out"][:ND].astype(np.float32)
                     for k in range(NC_CORES)])  # [8, 6250, H]
    out_full = outs.transpose(1, 0, 2).reshape(N, H)
    t4 = time.time()

    last_run_info.update({
        "exec_time_ns": res.exec_time_ns,
        "plan_s": t1 - t0, "build_s": t2 - t1, "run_s": t3 - t2,
        "unshard_s": t4 - t3, "fast_path": fast_path,
        "uniform_alpha": uniform_alpha,
        "NT": [NT], "trace": trace,
    })
    return out_full
